# revision 61
# speedup vs baseline: 48.9497x; 1.0098x over previous
"""Trainium2 Bass kernel for nn_EquiformerWEdgesBackbone.

Strategy (8 NeuronCores, SPMD, one compiled program):
  - Edges are sharded by DESTINATION node range: core d owns nodes
    [750*d, 750*d+750) and all edges whose dst falls in that range.
    Per-destination softmax and the segment-sum scatter are then fully
    core-local; the only collective is an AllGather of the per-node
    gather table (y = xn@Wv rows + xn0) once per layer -- each core
    builds only its OWN 750 table rows, the collective assembles the
    full 6000-row table.
  - Node state is kept TRANSPOSED in SBUF: x_T [C=128 partitions, (k, n)]
    so that every matmul (attention tables, FFN, output proj) can use it
    directly as the stationary operand without any per-tile transposes.
  - Activation discipline: ONLY functions from the `exp_and_others`
    ACT table set (exp, tanh, square, identity/copy) plus sqrt for the
    norms. sigmoid(x) = 0.5*(1+tanh(x/2)), silu(x) = x*sigmoid(x); the
    0.5 factors are folded into the downstream weight matrices on host.
    This avoids ~1283ns ACT-table reloads per function switch.
  - Attention logits:  pre_T[c',e] = Wa1c.T@e_T + Wa1a.T@xn0_src_T
    + per-tile one-hot matmul for the dst term; tanh-silu on ACT;
    logits/gate batched per 512-edge chunk into one PSUM tile, one
    strided Exp + one strided Tanh; softmax WITHOUT max subtraction
    (logits are O(0.5)); the division by the per-(node,head) sum is
    moved OUTSIDE the segment sum, so messages are scaled by
    w = exp(logits)*(1+tanh(gate/2)) only (2x folded into Wo).
  - Scatter: edges sorted by dst; per 128-node window the one-hot
    [edge,node] matmul accumulates agg[n, (h,k,v)] and the denominator
    sum (extra 8 columns) in PSUM across the window's edge tiles.
    One-hot matrices are DMAed per WINDOW (not per tile) from a host
    layout with 7KB contiguous rows.

kernel(**inputs) takes the FULL inputs, preprocesses indices on host
(sort/pad/one-hot -- no model math), compiles once, runs on cores 0-7,
and reassembles the full [N, K, C] float32 output.
"""

import os
import sys
import numpy as np
import ml_dtypes

sys.path.insert(0, "/opt/trn_rl_repo")

import concourse.bass as bass
import concourse.mybir as mybir
import concourse.tile as tile
from concourse import bacc
from concourse.bass_utils import run_bass_kernel_spmd

F32 = mybir.dt.float32
BF16 = mybir.dt.bfloat16
I16 = mybir.dt.int16
AF = mybir.ActivationFunctionType

# ---------------- problem constants (hardcoded) ----------------
N = 6000
E = 150000
K = 9
C = 128
H = 8
V = 16
L = 2
B = 512
RMAX = 5.0
EPS = 1e-6
GW = RMAX / B           # gaussian width
NCORES = 8
NLOC = N // NCORES      # 750 owned nodes per core
NLOCP = 768             # padded to 6 windows of 128
NWIN = NLOCP // 128
KNP = K * NLOCP         # 6912  x_T free size
YW = K * C              # 1152 y columns, (h,k,v) order
TROW = YW + C           # 1280 table row


def _cdiv(a, b):
    return (a + b - 1) // b


# ============================================================
# program builder
# ============================================================

def build_program(TW, mock_cc=False, reps=1):
    """Build the SPMD Bass program. TW = tiles (of 128 edge slots) per
    128-node window, uniform across cores and windows (any integer;
    windows are processed in groups of <=4 tiles).
    mock_cc replaces the AllGather with local DMAs (for single-core
    cost-model profiling only -- wrong results on >1 core)."""
    NT = NWIN * TW          # edge tiles per core
    ECAP = NT * 128         # edge slots per core
    ECAP_P = _cdiv(ECAP, 512) * 512   # P1 padding (512-chunks)
    NCH_P = ECAP_P // 512   # P1 chunks
    # per-window gather/compute groups: (tile offset, ntiles<=4)
    WGROUPS = [(4 * i, 4) for i in range(TW // 4)]
    if TW % 4:
        WGROUPS.append((TW - TW % 4, TW % 4))

    nc = bacc.Bacc("TRN2", target_bir_lowering=False, debug=False,
                   num_devices=NCORES)

    # ---------------- DRAM I/O ----------------
    def din(name, shape, dt):
        return nc.dram_tensor(name, shape, dt, kind="ExternalInput")

    # weights / constants (identical on all cores)
    wrbf1_d = din("wrbf1", [512, C], BF16)
    w2_d = din("wrbf2", [C, C], BF16)      # pre-scaled by 0.5 on host
    b1_d = din("brbf1", [1, C], F32)       # row layout (rank-1 bias matmul)
    b2_d = din("brbf2", [C, 1], F32)
    cneg_d = din("cneg", [C, 4], F32)          # -centers/GW per basis tile
    atomtab_d = din("atomtab", [40, C], F32)
    bondtt_d = din("bondtt", [C, 24], BF16)    # bond emb, transposed
    wa1_d = din("wa1", [L, 3, C, C], BF16)     # [l, {a,b,c}, c, c']
    wa2_d = din("wa2", [L, C, H], BF16)        # pre-scaled by 0.5
    wg_d = din("wg", [L, C, H], BF16)
    wv_d = din("wv", [L, C, C], BF16)          # columns in (h,v) order
    wo_d = din("wo", [L, C, C], BF16)          # rows (h,v); pre-scaled 0.5
    wf1_d = din("wf1", [L, C, 512], BF16)
    wf2_d = din("wf2", [L, 512, C], BF16)      # pre-scaled by 0.5

    QCH = _cdiv(NCH_P, 3)       # dist3 column groups

    # per-core data
    aoh_d = din("aoh", [40, NLOCP], F32)       # atom one-hot (cols = own nodes)
    d3_d = din("dist3", [3, QCH * 512], F32)   # distances, chunk ch at
    #                         row ch%3 (-> partition 32*(ch%3)), col ch//3
    boh_d = din("boh", [24, ECAP_P], BF16)       # bond one-hot (transposed)
    # one-hots batched per window: [w, lane, tile*128 + other]
    ohen_d = din("ohen", [NWIN, 128, TW * 128], BF16)  # lane=edge, col=node
    ohne_d = din("ohne", [NWIN, 128, TW * 128], BF16)  # lane=node, col=edge
    idx_d = din("idx", [128, NT * 8], I16)  # wrapped gather indices

    out_d = nc.dram_tensor("xout", [C, KNP], BF16, kind="ExternalOutput")

    # internal DRAM: per-layer gather table (own rows -> allgathered full)
    aginy = nc.dram_tensor("aginy", [NLOCP, TROW], BF16, kind="Internal")
    agouty = nc.dram_tensor("agouty", [NCORES * NLOCP, TROW], BF16,
                            kind="Internal", addr_space="Shared")

    with tile.TileContext(nc) as tc:
        import contextlib
        ctx = contextlib.ExitStack()
        with ctx:
            wp = ctx.enter_context(tc.tile_pool(name="wp", bufs=1))
            big = ctx.enter_context(tc.tile_pool(name="big", bufs=1))

            # ---------- load weights to SBUF ----------
            def wtile(name, shape, dt, src_ap):
                t = wp.tile(shape, dt, tag=name)
                nc.sync.dma_start(out=t[:], in_=src_ap)
                return t

            w1t = wtile("w1t", [128, 4, C], BF16,
                        wrbf1_d.ap().rearrange("(t b) c -> b t c", b=128))
            w2t = wtile("w2t", [C, C], BF16, w2_d.ap())
            b1t = wtile("b1t", [1, C], F32, b1_d.ap())
            b2t = wtile("b2t", [C, 1], F32, b2_d.ap())
            cnegt = wtile("cnegt", [C, 4], F32, cneg_d.ap())
            atomt = wtile("atomt", [40, C], F32, atomtab_d.ap())
            bondtt = wtile("bondtt", [C, 24], BF16, bondtt_d.ap())
            wa1t = wtile("wa1t", [C, L, 3, C], BF16,
                         wa1_d.ap().rearrange("l t c d -> c l t d"))
            wa2t = wtile("wa2t", [C, L, H], BF16,
                         wa2_d.ap().rearrange("l c h -> c l h"))
            wgt = wtile("wgt", [C, L, H], BF16,
                        wg_d.ap().rearrange("l c h -> c l h"))
            wvt = wtile("wvt", [C, L, C], BF16,
                        wv_d.ap().rearrange("l c d -> c l d"))
            wot = wtile("wot", [C, L, C], BF16,
                        wo_d.ap().rearrange("l c d -> c l d"))
            wf1t = wtile("wf1t", [C, L, 512], BF16,
                         wf1_d.ap().rearrange("l c f -> c l f"))
            wf2t = wtile("wf2t", [128, L, 4, C], BF16,
                         wf2_d.ap().rearrange("l (t b) c -> b l t c", b=128))

            identt = wp.tile([128, 128], BF16, tag="identt")
            from concourse.masks import make_identity
            make_identity(nc, identt[:])
            ones1 = wp.tile([1, 128], F32, tag="ones1")
            nc.vector.memset(ones1[:], 1.0)
            ones128 = wp.tile([128, 1], BF16, tag="ones128")
            nc.vector.memset(ones128[:], 1.0)
            ones512 = wp.tile([1, 512], F32, tag="ones512")
            nc.vector.memset(ones512[:], 1.0)
            ones65 = wp.tile([65, 128], F32, tag="ones65")
            nc.vector.memset(ones65[:], 1.0)
            epst = wp.tile([1, 1], F32, tag="epst")
            nc.vector.memset(epst[:], float(EPS))

            # bond-embedding contributions to the attention pre-act and the
            # gate, folded through the (linear) edge-feature path:
            #   bw[l] = bond_emb @ Wa1c[l]   [24, C]
            #   bg[l] = bond_emb @ Wg[l]     [24, H]
            bwt = wp.tile([24, L, C], BF16, tag="bwt")
            bgt = wp.tile([24, L, H], BF16, tag="bgt")
            with tc.tile_pool(name="bwps", bufs=2, space="PSUM") as bwps:
                for l in range(L):
                    ps = bwps.tile([24, C], F32, tag="bwp")
                    nc.tensor.matmul(ps[:], lhsT=bondtt[:],
                                     rhs=wa1t[:, l, 2, :],
                                     start=True, stop=True)
                    nc.scalar.activation(out=bwt[:, l, :], in_=ps[:],
                                         func=AF.Identity)
                    ps2 = bwps.tile([24, H], F32, tag="bgp")
                    nc.tensor.matmul(ps2[:], lhsT=bondtt[:],
                                     rhs=wgt[:, l, :],
                                     start=True, stop=True)
                    nc.scalar.activation(out=bgt[:, l, :], in_=ps2[:],
                                         func=AF.Identity)

            # persistent SBUF state
            x_T = big.tile([C, KNP], F32, tag="x_T")
            e_T = big.tile([C, ECAP_P], BF16, tag="e_T")
            idxs = big.tile([128, NT * 8], I16, tag="idxs")
            nc.sync.dma_start(out=idxs[:], in_=idx_d.ap())
            xn_bf = big.tile([C, KNP], BF16, tag="xn_bf")
            scr = big.tile([C, KNP], BF16, tag="scr")    # x^2 / final out
            t1w = big.tile([128, NWIN, C], BF16, tag="t1w")
            sig0 = big.tile([128, 4, NLOCP], BF16, tag="sig0")
            ssk = big.tile([1, NLOCP], F32, tag="ssk")
            rs = big.tile([1, NLOCP], F32, tag="rs")

            for _rep in range(reps):
                # ---------- P0: init x_T (atom embeddings into l=0) ----------
                with tc.tile_pool(name="p0ps", bufs=2, space="PSUM") as p0ps, \
                     tc.tile_pool(name="p0sb", bufs=1) as p0sb:
                    nc.vector.memset(x_T[:], 0.0)
                    aohs = p0sb.tile([40, NLOCP], F32, tag="aohs")
                    nc.sync.dma_start(out=aohs[:], in_=aoh_d.ap())
                    for j in range(2):  # two chunks of 384 node cols
                        ps = p0ps.tile([C, 384], F32, tag="a0")
                        nc.tensor.matmul(ps[:], lhsT=atomt[:],
                                         rhs=aohs[:, j * 384:(j + 1) * 384],
                                         start=True, stop=True)
                        nc.vector.tensor_copy(
                            out=x_T[:, j * 384:(j + 1) * 384], in_=ps[:])

                # ---------- helper: rms_sh normalize x_T -> dst ----------
                def rms_norm(dst_tile, dst_dt):
                    with tc.tile_pool(name="rmps", bufs=2, space="PSUM") as rmps:
                        nc.scalar.activation(out=scr[:], in_=x_T[:],
                                             func=AF.Square)
                        # sum over c (partitions, via ones matmul) AND k
                        # (PSUM accumulation across the 9 k-blocks).
                        # NB: each matmul output must stay inside one PSUM
                        # bank (512 f32), so split 768 as 512+256.
                        ps = rmps.tile([1, 1024], F32, tag="ss")
                        for k in range(K):
                            for j0, j1 in ((0, 512), (512, NLOCP)):
                                nc.tensor.matmul(
                                    ps[:, j0:j1], lhsT=ones128[:],
                                    rhs=scr[:, k * NLOCP + j0:k * NLOCP + j1],
                                    start=(k == 0), stop=(k == K - 1))
                        nc.vector.tensor_copy(out=ssk[:], in_=ps[:, 0:NLOCP])
                        nc.scalar.activation(out=rs[:], in_=ssk[:], func=AF.Sqrt,
                                             scale=1.0 / (K * C),
                                             bias=epst[:, 0:1])
                        nc.vector.reciprocal(out=rs[:], in_=rs[:])
                        rb = rmps.tile([C, 1024], F32, tag="rb")
                        nc.tensor.matmul(rb[:, 0:512], lhsT=ones1[:],
                                         rhs=rs[:, 0:512], start=True, stop=True)
                        nc.tensor.matmul(rb[:, 512:NLOCP], lhsT=ones1[:],
                                         rhs=rs[:, 512:NLOCP],
                                         start=True, stop=True)
                        nc.vector.tensor_tensor(
                            out=dst_tile[:].rearrange("c (k n) -> c k n", k=K),
                            in0=x_T[:].rearrange("c (k n) -> c k n", k=K),
                            in1=bass.AP(tensor=rb[:].tensor, offset=rb[:].offset,
                                        ap=[rb[:].ap[0], [0, K], [1, NLOCP]]),
                            op=mybir.AluOpType.mult)

                # ---------- helper: build OWN table rows + allgather ----------
                def table_allgather(l):
                    with tc.tile_pool(name="tbps", bufs=2, space="PSUM") as tbps, \
                         tc.tile_pool(name="tbsb", bufs=2) as tbsb:
                        for j in range(NWIN):
                            cn = 128
                            j0 = j * 128
                            ysb = tbsb.tile([128, TROW], BF16, tag="ysb")
                            for k in range(K):
                                ps = tbps.tile([128, C], F32, tag="yp")
                                nc.tensor.matmul(
                                    ps[:cn, :],
                                    lhsT=xn_bf[:, k * NLOCP + j0:
                                               k * NLOCP + j0 + cn],
                                    rhs=wvt[:, l, :], start=True, stop=True)
                                # scatter into (h, k, v) column order (on ACT
                                # -- DVE is the busier engine here)
                                nc.scalar.activation(
                                    out=bass.AP(
                                        tensor=ysb[:].tensor,
                                        offset=ysb[:].offset + k * V,
                                        ap=[ysb[:].ap[0], [K * V, H], [1, V]]
                                    )[:cn],
                                    in_=ps[:cn, :].rearrange(
                                        "n (h v) -> n h v", h=H),
                                    func=AF.Identity)
                            tp = tbps.tile([128, C], BF16, tag="x0t")
                            nc.tensor.transpose(
                                out=tp[:cn, :], in_=xn_bf[:, j0:j0 + cn],
                                identity=identt[:])
                            nc.scalar.activation(out=ysb[:cn, YW:TROW],
                                                 in_=tp[:cn, :],
                                                 func=AF.Identity)
                            nc.sync.dma_start(
                                out=aginy.ap()[j0:j0 + cn, :],
                                in_=ysb[:cn, :])
                    if mock_cc:
                        for dd in range(NCORES):
                            nc.sync.dma_start(
                                out=agouty.ap()[dd * NLOCP:(dd + 1) * NLOCP, :],
                                in_=aginy.ap())
                    else:
                        nc.gpsimd.collective_compute(
                            "AllGather", mybir.AluOpType.bypass,
                            replica_groups=[list(range(NCORES))],
                            ins=[aginy.ap()], outs=[agouty.ap()])

                # ---------- P1: edge features  e_T [c, ECAP] ----------
                # Runs entirely DMA-free (distances broadcast from the
                # preloaded distt tile via a PE ones-matmul) so it can
                # overlap the first AllGather, which hogs the DMA rings.
                def edge_features(distt, c0, c1):
                    with tc.tile_pool(name="p1ps", bufs=2, space="PSUM") as p1ps, \
                         tc.tile_pool(name="p1sb", bufs=3) as p1sb:
                        for ch in range(c0, c1):
                            sl = slice(ch * 512, (ch + 1) * 512)
                            pp, q3 = 32 * (ch % 3), ch // 3
                            dbp = p1ps.tile([128, 512], F32, tag="dbp")
                            nc.tensor.matmul(
                                dbp[:], lhsT=ones65[pp:pp + 1, :],
                                rhs=distt[pp:pp + 1,
                                          q3 * 512:(q3 + 1) * 512],
                                start=True, stop=True)
                            h1 = p1ps.tile([C, 512], F32, tag="h1")
                            for bt in range(4):
                                # split the squares ACT/DVE to balance
                                # engines in this ACT-heavy phase
                                sq = p1sb.tile([128, 512], F32, tag="sq")
                                if bt % 2 == 0:
                                    nc.scalar.activation(
                                        out=sq[:], in_=dbp[:],
                                        func=AF.Square,
                                        bias=cnegt[:, bt:bt + 1],
                                        scale=1.0 / GW)
                                else:
                                    sh = p1sb.tile([128, 512], F32, tag="sh")
                                    nc.vector.tensor_scalar(
                                        out=sh[:], in0=dbp[:],
                                        scalar1=1.0 / GW,
                                        scalar2=cnegt[:, bt:bt + 1],
                                        op0=mybir.AluOpType.mult,
                                        op1=mybir.AluOpType.add)
                                    nc.vector.tensor_tensor(
                                        out=sq[:], in0=sh[:], in1=sh[:],
                                        op=mybir.AluOpType.mult)
                                rbf = p1sb.tile([128, 512], BF16, tag="rbf")
                                nc.scalar.activation(out=rbf[:], in_=sq[:],
                                                     func=AF.Exp, scale=-1.0)
                                nc.tensor.matmul(h1[:], lhsT=w1t[:, bt, :],
                                                 rhs=rbf[:],
                                                 start=(bt == 0), stop=False)
                            # + b1 (rank-1) so silu sees h1 + b1
                            nc.tensor.matmul(h1[:], lhsT=b1t[:], rhs=ones512[:],
                                             start=False, stop=True)
                            # hs = 2*silu(h1) = h1 + h1*tanh(h1/2), all bf16
                            th = p1sb.tile([C, 512], BF16, tag="th")
                            nc.scalar.activation(out=th[:], in_=h1[:],
                                                 func=AF.Tanh, scale=0.5)
                            h1b = p1sb.tile([C, 512], BF16, tag="h1b")
                            nc.scalar.activation(out=h1b[:], in_=h1[:],
                                                 func=AF.Identity)
                            u1 = p1sb.tile([C, 512], BF16, tag="u1")
                            nc.vector.tensor_tensor(out=u1[:], in0=h1b[:],
                                                    in1=th[:],
                                                    op=mybir.AluOpType.mult)
                            hs = p1sb.tile([C, 512], BF16, tag="hs")
                            nc.vector.tensor_tensor(out=hs[:], in0=h1b[:],
                                                    in1=u1[:],
                                                    op=mybir.AluOpType.add)
                            ep = p1ps.tile([C, 512], F32, tag="ep")
                            nc.tensor.matmul(ep[:], lhsT=w2t[:], rhs=hs[:],
                                             start=True, stop=True)
                            nc.vector.tensor_scalar_add(
                                out=e_T[:, sl], in0=ep[:],
                                scalar1=b2t[:, 0:1])

                # ---------- layer 0 prologue ----------
                # P1 head overlaps P0/rms/ybuild; P1 tail overlaps the
                # first AllGather (P1 is DMA-free; a collective blocks
                # all other DMA traffic)
                with tc.tile_pool(name="distp", bufs=1) as distp:
                    distt = distp.tile([65, QCH * 512], F32, tag="distt")
                    for r3 in range(3):
                        nc.sync.dma_start(out=distt[32 * r3:32 * r3 + 1, :],
                                          in_=d3_d.ap()[r3:r3 + 1, :])
                    P1H = 6
                    edge_features(distt, 0, P1H)
                    rms_norm(xn_bf, BF16)
                    table_allgather(0)
                    edge_features(distt, P1H, NCH_P)

                # ---------- per layer ----------
                for l in range(L):
                    if l > 0:
                        rms_norm(xn_bf, BF16)
                        table_allgather(l)

                    # t1w per window: [n, c'] = xn0_win.T @ Wa1b
                    with tc.tile_pool(name="t1ps", bufs=2, space="PSUM") as t1ps:
                        for w in range(NWIN):
                            ps = t1ps.tile([128, C], F32, tag="t1")
                            nc.tensor.matmul(
                                ps[:], lhsT=xn_bf[:, w * 128:(w + 1) * 128],
                                rhs=wa1t[:, l, 1, :], start=True, stop=True)
                            nc.vector.tensor_copy(out=t1w[:, w, :], in_=ps[:])

                    # ---------- edge loop ----------
                    with tc.tile_pool(name="agps", bufs=1, space="PSUM") as agps, \
                         tc.tile_pool(name="pps", bufs=2, space="PSUM") as pps, \
                         tc.tile_pool(name="lps", bufs=1, space="PSUM") as lps, \
                         tc.tile_pool(name="mps", bufs=1, space="PSUM") as mps, \
                         tc.tile_pool(name="esb", bufs=3) as esb, \
                         tc.tile_pool(name="episb", bufs=1) as episb, \
                         tc.tile_pool(name="ohp", bufs=2) as ohp, \
                         tc.tile_pool(name="gsb", bufs=2) as gsb:
                        for w in range(NWIN):
                            ohenw = ohp.tile([128, TW, 128], BF16, tag="ohenw")
                            nc.sync.dma_start(
                                out=ohenw[:],
                                in_=ohen_d.ap()[w].rearrange(
                                    "p (t e) -> p t e", t=TW))
                            ohnew = ohp.tile([128, TW, 128], BF16, tag="ohnew")
                            nc.sync.dma_start(
                                out=ohnew[:],
                                in_=ohne_d.ap()[w].rearrange(
                                    "p (t e) -> p t e", t=TW))
                            # sden rides in agg's third PSUM bank (cols
                            # 1152:1160) to free a bank for pre double-buf
                            agg = agps.tile([128, YW + H], F32, tag="agg")
                            sden = agg[:, YW:YW + H]
                            for toff, ntl in WGROUPS:
                                t0g = w * TW + toff         # global tile
                                e0 = t0g * 128              # global edge col
                                ne = ntl * 128
                                ybuf = gsb.tile([128, 4, YW], BF16,
                                                tag="ybuf")
                                nc.gpsimd.dma_gather(
                                    ybuf[:, 0:ntl], agouty.ap()[:, 0:YW],
                                    idxs[:, t0g * 8:(t0g + ntl) * 8],
                                    ne, ne, YW, elem_step=TROW)
                                x0b = gsb.tile([128, 1, ne], BF16,
                                               tag=f"x0b{ntl}")
                                nc.gpsimd.dma_gather(
                                    x0b[:], agouty.ap()[:, YW:TROW],
                                    idxs[:, t0g * 8:(t0g + ntl) * 8],
                                    ne, ne, C, elem_step=TROW,
                                    transpose=True)
                                bohc = esb.tile([24, 512], BF16, tag="bohc")
                                nc.sync.dma_start(
                                    out=bohc[:, 0:ne],
                                    in_=boh_d.ap()[:, e0:e0 + ne])
                                pre = pps.tile([C, 512], F32, tag="pre")
                                nc.tensor.matmul(
                                    pre[:, 0:ne], lhsT=wa1t[:, l, 2, :],
                                    rhs=e_T[:, e0:e0 + ne],
                                    start=True, stop=False)
                                nc.tensor.matmul(
                                    pre[:, 0:ne], lhsT=bwt[:, l, :],
                                    rhs=bohc[:, 0:ne],
                                    start=False, stop=False)
                                nc.tensor.matmul(
                                    pre[:, 0:ne], lhsT=wa1t[:, l, 0, :],
                                    rhs=x0b[:, 0, :], start=False, stop=False)
                                for sub in range(ntl):
                                    tt = toff + sub
                                    nc.tensor.matmul(
                                        pre[:, sub * 128:(sub + 1) * 128],
                                        lhsT=t1w[:, w, :], rhs=ohnew[:, tt, :],
                                        start=False, stop=(sub == ntl - 1))
                                # preS = 2*silu(pre) = pre + pre*tanh(pre/2)
                                thp = esb.tile([C, 512], BF16, tag="thp")
                                nc.scalar.activation(out=thp[:, 0:ne],
                                                     in_=pre[:, 0:ne],
                                                     func=AF.Tanh, scale=0.5)
                                preb = esb.tile([C, 512], BF16, tag="preb")
                                nc.scalar.activation(out=preb[:, 0:ne],
                                                     in_=pre[:, 0:ne],
                                                     func=AF.Identity)
                                up = esb.tile([C, 512], BF16, tag="up")
                                nc.vector.tensor_tensor(
                                    out=up[:, 0:ne], in0=preb[:, 0:ne],
                                    in1=thp[:, 0:ne],
                                    op=mybir.AluOpType.mult)
                                preS = esb.tile([C, 512], BF16, tag="preS")
                                nc.vector.tensor_tensor(
                                    out=preS[:, 0:ne], in0=preb[:, 0:ne],
                                    in1=up[:, 0:ne],
                                    op=mybir.AluOpType.add)
                                # ---- logits/gate batched over the group ----
                                lgc = lps.tile([128, 4, 2 * H], F32, tag="lgc")
                                for sub in range(ntl):
                                    nc.tensor.matmul(
                                        lgc[:, sub, 0:H],
                                        lhsT=preS[:, sub * 128:(sub + 1) * 128],
                                        rhs=wa2t[:, l, :],
                                        start=True, stop=True)
                                    nc.tensor.matmul(
                                        lgc[:, sub, H:2 * H],
                                        lhsT=e_T[:, e0 + sub * 128:
                                                 e0 + (sub + 1) * 128],
                                        rhs=wgt[:, l, :],
                                        start=True, stop=False)
                                    nc.tensor.matmul(
                                        lgc[:, sub, H:2 * H],
                                        lhsT=bohc[:, sub * 128:(sub + 1) * 128],
                                        rhs=bgt[:, l, :],
                                        start=False, stop=True)
                                exc = esb.tile([128, 4, H], BF16, tag="exc")
                                nc.scalar.activation(out=exc[:, 0:ntl, :],
                                                     in_=lgc[:, 0:ntl, 0:H],
                                                     func=AF.Exp)
                                gtc = esb.tile([128, 4, H], BF16, tag="gtc")
                                nc.scalar.activation(
                                    out=gtc[:, 0:ntl, :],
                                    in_=lgc[:, 0:ntl, H:2 * H],
                                    func=AF.Tanh, scale=0.5)
                                nc.vector.tensor_scalar_add(
                                    out=gtc[:, 0:ntl, :],
                                    in0=gtc[:, 0:ntl, :], scalar1=1.0)
                                # wb duplicated in adjacent pairs so the msk
                                # multiply qualifies for the packed DVE mode
                                wbc2 = esb.tile([128, 4, H, 2], BF16,
                                                tag="wbc2")
                                nc.vector.tensor_tensor(
                                    out=bass.AP(
                                        tensor=wbc2[:].tensor,
                                        offset=wbc2[:].offset,
                                        ap=[wbc2[:].ap[0], [2, ntl * H],
                                            [1, 2]]),
                                    in0=bass.AP(
                                        tensor=exc[:].tensor,
                                        offset=exc[:].offset,
                                        ap=[exc[:].ap[0], [1, ntl * H],
                                            [0, 2]]),
                                    in1=bass.AP(
                                        tensor=gtc[:].tensor,
                                        offset=gtc[:].offset,
                                        ap=[gtc[:].ap[0], [1, ntl * H],
                                            [0, 2]]),
                                    op=mybir.AluOpType.mult)
                                # ---- per-tile: scale + scatter ----
                                for sub in range(ntl):
                                    tt = toff + sub
                                    msk = esb.tile([128, YW], BF16, tag="msk")
                                    nc.vector.tensor_tensor(
                                        out=bass.AP(
                                            tensor=msk[:].tensor,
                                            offset=msk[:].offset,
                                            ap=[msk[:].ap[0], [K * V, H],
                                                [2, K * V // 2], [1, 2]]),
                                        in0=bass.AP(
                                            tensor=ybuf[:].tensor,
                                            offset=(ybuf[:].offset
                                                    + sub * YW),
                                            ap=[ybuf[:].ap[0], [K * V, H],
                                                [2, K * V // 2], [1, 2]]),
                                        in1=bass.AP(
                                            tensor=wbc2[:].tensor,
                                            offset=wbc2[:].offset + sub * 2 * H,
                                            ap=[wbc2[:].ap[0], [2, H],
                                                [0, K * V // 2], [1, 2]]),
                                        op=mybir.AluOpType.mult)
                                    st = (tt == 0)
                                    sp = (tt == TW - 1)
                                    nc.tensor.matmul(agg[:, 0:512],
                                                     lhsT=ohenw[:, tt, :],
                                                     rhs=msk[:, 0:512],
                                                     start=st, stop=sp)
                                    nc.tensor.matmul(agg[:, 512:1024],
                                                     lhsT=ohenw[:, tt, :],
                                                     rhs=msk[:, 512:1024],
                                                     start=st, stop=sp)
                                    nc.tensor.matmul(agg[:, 1024:YW],
                                                     lhsT=ohenw[:, tt, :],
                                                     rhs=msk[:, 1024:YW],
                                                     start=st, stop=sp)
                                    nc.tensor.matmul(sden,
                                                     lhsT=ohenw[:, tt, :],
                                                     rhs=exc[:, sub, :],
                                                     start=st, stop=sp)
                            # ----- window epilogue -----
                            rcp = episb.tile([128, H], F32, tag="rcp")
                            nc.vector.tensor_scalar_add(out=rcp[:], in0=sden,
                                                        scalar1=1e-9)
                            nc.vector.reciprocal(out=rcp[:], in_=rcp[:])
                            aggn = episb.tile([128, YW], BF16, tag="aggn")
                            nc.vector.tensor_tensor(
                                out=bass.AP(
                                    tensor=aggn[:].tensor,
                                    offset=aggn[:].offset,
                                    ap=[aggn[:].ap[0], [1, K * V], [K * V, H]]),
                                in0=bass.AP(
                                    tensor=agg[:].tensor,
                                    offset=agg[:].offset,
                                    ap=[agg[:].ap[0], [1, K * V], [K * V, H]]),
                                in1=bass.AP(
                                    tensor=rcp[:].tensor,
                                    offset=rcp[:].offset,
                                    ap=[rcp[:].ap[0], [0, K * V], [1, H]]),
                                op=mybir.AluOpType.mult)
                            agr = episb.tile([128, YW], BF16, tag="agr")
                            nc.scalar.activation(
                                out=agr[:].rearrange("n (k h v) -> n k h v",
                                                     k=K, h=H),
                                in_=aggn[:].rearrange("n (h k v) -> n k h v",
                                                      h=H, k=K),
                                func=AF.Identity)
                            for k in range(K):
                                tp = mps.tile([128, 128], BF16, tag="atp")
                                nc.tensor.transpose(
                                    out=tp[:],
                                    in_=agr[:, k * 128:(k + 1) * 128],
                                    identity=identt[:])
                                aT = esb.tile([128, 128], BF16, tag="aT")
                                nc.scalar.activation(out=aT[:], in_=tp[:],
                                                     func=AF.Identity)
                                dk = mps.tile([128, 128], F32, tag="dk")
                                nc.tensor.matmul(dk[:], lhsT=wot[:, l, :],
                                                 rhs=aT[:], start=True, stop=True)
                                nc.vector.tensor_tensor(
                                    out=x_T[:, k * NLOCP + w * 128:
                                            k * NLOCP + (w + 1) * 128],
                                    in0=x_T[:, k * NLOCP + w * 128:
                                            k * NLOCP + (w + 1) * 128],
                                    in1=dk[:], op=mybir.AluOpType.add)

                    # ---------- FFN ----------
                    rms_norm(xn_bf, BF16)
                    with tc.tile_pool(name="fps", bufs=2, space="PSUM") as fps, \
                         tc.tile_pool(name="fsb", bufs=3) as fsb:
                        # col chunks of 384: 18 chunks; chunks 0,1 are k=0
                        for j in range(18):
                            c0 = j * 384
                            dlt = fps.tile([C, 384], F32, tag="dlt")
                            for fc in range(4):
                                hp = fps.tile([128, 384], F32, tag="hp")
                                nc.tensor.matmul(
                                    hp[:], lhsT=wf1t[:, l, fc * 128:(fc + 1) * 128],
                                    rhs=xn_bf[:, c0:c0 + 384],
                                    start=True, stop=True)
                                hpb = fsb.tile([128, 384], BF16, tag="hpb")
                                nc.scalar.activation(out=hpb[:], in_=hp[:],
                                                     func=AF.Identity)
                                hb = fsb.tile([128, 384], BF16, tag="hb")
                                if j < 2:
                                    # sig0 = 1 + tanh(hp/2) = 2*sigmoid(hp)
                                    sl0 = sig0[:, fc, j * 384:(j + 1) * 384]
                                    nc.scalar.activation(
                                        out=sl0, in_=hp[:],
                                        func=AF.Tanh, scale=0.5)
                                    nc.vector.tensor_scalar_add(
                                        out=sl0, in0=sl0, scalar1=1.0)
                                    nc.vector.tensor_tensor(
                                        out=hb[:], in0=hpb[:], in1=sl0,
                                        op=mybir.AluOpType.mult)
                                else:
                                    nsl = slice((j % 2) * 384, (j % 2) * 384 + 384)
                                    nc.vector.tensor_tensor(
                                        out=hb[:], in0=hpb[:],
                                        in1=sig0[:, fc, nsl],
                                        op=mybir.AluOpType.mult)
                                nc.tensor.matmul(
                                    dlt[:], lhsT=wf2t[:, l, fc, :], rhs=hb[:],
                                    start=(fc == 0), stop=(fc == 3))
                            nc.vector.tensor_tensor(
                                out=x_T[:, c0:c0 + 384],
                                in0=x_T[:, c0:c0 + 384],
                                in1=dlt[:], op=mybir.AluOpType.add)

                # ---------- final norm + output ----------
                rms_norm(scr, BF16)
                nc.sync.dma_start(out=out_d.ap(), in_=scr[:])

    nc.compile()
    return nc


# ============================================================
# host preprocessing + runner
# ============================================================

_CACHE = {}


_PERMS = {}


def _prep(inputs):
    """Index-only host preprocessing; returns (TW, per-core in_maps)."""
    atom_feats = np.asarray(inputs["atom_feats"]).astype(np.int64)
    bond_feats = np.asarray(inputs["bond_feats"]).astype(np.int64)
    edge_index = np.asarray(inputs["edge_index"]).astype(np.int64)
    edge_distance = np.asarray(inputs["edge_distance"]).astype(np.float32)

    src, dst = edge_index[0], edge_index[1]

    # ---- per-core edge partition by dst range ----
    # Nodes are rebalanced into the 6 windows (greedy by degree) to
    # minimize the max per-window edge count -> smaller TW.  The window
    # assignment is a pure host-side permutation: table rows, one-hots,
    # gather indices and the output unpack all use permuted positions.
    cores = []
    perms = []          # perms[d][l] = permuted position p in [0, NLOCP)
    maxcnt = 0
    for d in range(NCORES):
        sel = np.nonzero((dst >= d * NLOC) & (dst < (d + 1) * NLOC))[0]
        dl = (dst[sel] - d * NLOC).astype(np.int64)
        deg = np.bincount(dl, minlength=NLOC)
        order = np.argsort(-deg, kind="stable")
        wload = np.zeros(NWIN, np.int64)
        wslots = np.zeros(NWIN, np.int64)
        perm = np.zeros(NLOC, np.int64)
        for l in order:
            cand = np.nonzero(wslots < 128)[0]
            w = cand[np.argmin(wload[cand])]
            perm[l] = w * 128 + wslots[w]
            wslots[w] += 1
            wload[w] += deg[l]
        pp = perm[dl]                       # permuted position per edge
        eorder = np.argsort(pp, kind="stable")
        sel = sel[eorder]
        pp = pp[eorder]
        cnts = np.bincount(pp // 128, minlength=NWIN)
        maxcnt = max(maxcnt, int(cnts.max()))
        cores.append((sel, pp, cnts))
        perms.append(perm)
    TW = _cdiv(maxcnt, 128)
    NT = NWIN * TW
    ECAP = NT * 128
    ECAP_P = _cdiv(ECAP, 512) * 512         # e_T/bond/dist padding
    NCH_P = ECAP_P // 512
    _PERMS["perms"] = perms

    # ---- weights ----
    f32 = np.float32
    bf16 = ml_dtypes.bfloat16
    W_rbf1 = np.asarray(inputs["W_rbf1"], f32)
    W_rbf2 = np.asarray(inputs["W_rbf2"], f32)
    b_rbf1 = np.asarray(inputs["b_rbf1"], f32)
    b_rbf2 = np.asarray(inputs["b_rbf2"], f32)
    Wa1 = np.asarray(inputs["Wa1"], f32)
    Wa2 = np.asarray(inputs["Wa2"], f32)
    Wv = np.asarray(inputs["Wv"], f32)
    Wg = np.asarray(inputs["Wg"], f32)
    Wo = np.asarray(inputs["Wo"], f32)
    Wf1 = np.asarray(inputs["Wf1"], f32)
    Wf2 = np.asarray(inputs["Wf2"], f32)
    atom_emb = np.asarray(inputs["atom_emb"], f32)
    bond_emb = np.asarray(inputs["bond_emb"], f32)

    centers = np.linspace(0.0, RMAX, B).astype(f32)
    cneg = (-centers / GW).reshape(4, 128).T.copy()       # [128, 4]

    common = {
        "wrbf1": W_rbf1.astype(bf16),
        # tanh-silu emits 2*silu: fold the 0.5 into the next matmul
        "wrbf2": (0.5 * W_rbf2).astype(bf16),
        "brbf1": b_rbf1.reshape(1, C),
        "brbf2": b_rbf2.reshape(C, 1),
        "cneg": np.ascontiguousarray(cneg),
        "atomtab": atom_emb.reshape(40, C).copy(),
        "bondtt": np.ascontiguousarray(
            bond_emb.reshape(24, C).T).astype(bf16),
        "wa1": np.ascontiguousarray(
            Wa1.reshape(L, 3, C, C)).astype(bf16),
        "wa2": (0.5 * Wa2).astype(bf16),
        "wg": Wg.astype(bf16),
        "wv": Wv.astype(bf16),
        "wo": (0.5 * Wo).astype(bf16),
        "wf1": Wf1.astype(bf16),
        "wf2": (0.5 * Wf2).astype(bf16),
    }

    # global table row of node n (permuted, NLOCP rows per core)
    g_tabrow = np.zeros(N, np.int64)
    for d in range(NCORES):
        g_tabrow[d * NLOC:(d + 1) * NLOC] = d * NLOCP + perms[d]

    in_maps = []
    for d in range(NCORES):
        sel, pp, cnts = cores[d]
        # slot layout: window w occupies tiles [w*TW, (w+1)*TW)
        slot_src = np.zeros(ECAP, np.int64)
        slot_dln = np.zeros(ECAP, np.int64)   # dst-in-window (permuted)
        slot_valid = np.zeros(ECAP, bool)
        pos = 0
        for w in range(NWIN):
            cnt = int(cnts[w])
            base = w * TW * 128
            slot_src[base:base + cnt] = g_tabrow[src[sel[pos:pos + cnt]]]
            slot_dln[base:base + cnt] = pp[pos:pos + cnt] - w * 128
            slot_valid[base:base + cnt] = True
            pos += cnt

        # one-hot [edge, node] per tile (+ transpose)
        ohen = np.zeros((NT, 128, 128), bf16)
        tl = np.arange(ECAP)
        tn, te = tl // 128, tl % 128
        v = slot_valid
        ohen[tn[v], te[v], slot_dln[v]] = 1.0
        ohne = np.ascontiguousarray(ohen.transpose(0, 2, 1))
        # window-batched layouts: [NWIN, 128, TW*128]
        ohenw = np.ascontiguousarray(
            ohen.reshape(NWIN, TW, 128, 128).transpose(0, 2, 1, 3)
                .reshape(NWIN, 128, TW * 128))
        ohnew = np.ascontiguousarray(
            ohne.reshape(NWIN, TW, 128, 128).transpose(0, 2, 1, 3)
                .reshape(NWIN, 128, TW * 128))

        # distances / bond one-hot (padded to 512-chunk multiple for P1)
        dist = np.zeros(ECAP_P, f32)
        dist[np.nonzero(slot_valid)[0]] = edge_distance[sel]
        QCH = _cdiv(NCH_P, 3)
        dist3 = np.zeros((3, QCH * 512), f32)
        for ch in range(NCH_P):
            dist3[ch % 3, (ch // 3) * 512:(ch // 3 + 1) * 512] = \
                dist[ch * 512:(ch + 1) * 512]
        boh = np.zeros((24, ECAP_P), f32)
        for f in range(3):
            boh[f * 8 + bond_feats[sel, f], np.nonzero(slot_valid)[0]] = 1.0

        # atom one-hot (own nodes at permuted cols)
        aoh = np.zeros((40, NLOCP), f32)
        own = np.arange(d * NLOC, (d + 1) * NLOC)
        for f in range(4):
            aoh[f * 10 + atom_feats[own, f], perms[d]] = 1.0

        # wrapped int16 gather indices (per-tile wrapping == per-gather
        # wrapping for any run of whole tiles, since 128 % 16 == 0)
        idx = np.zeros((16, NT * 8), np.int16)
        g = np.arange(ECAP)
        ti, ei = g // 128, g % 128
        idx[ei % 16, ti * 8 + ei // 16] = slot_src.astype(np.int16)
        idx = np.tile(idx, (8, 1))

        m = dict(common)
        m.update({
            "aoh": aoh, "dist3": dist3, "boh": boh.astype(bf16),
            "ohen": ohenw, "ohne": ohnew, "idx": idx,
        })
        in_maps.append(m)
    return TW, in_maps


def _get_nc(TW, reps=1):
    key = ('nc', TW, reps)
    if key not in _CACHE:
        _CACHE[key] = build_program(TW, reps=reps)
    return _CACHE[key]


def _make_runner(nc, reps):
    """jit-compiled SPMD runner that chains the NEFF `reps` times
    back-to-back (outputs fed back as the donated output operands), so
    (T(reps) - T(1)) / (reps - 1) cancels host/axon dispatch overhead."""
    import jax
    from jax.sharding import Mesh, PartitionSpec
    from jax.experimental.shard_map import shard_map
    from concourse import bass2jax
    import concourse.mybir as mb

    bass2jax.install_neuronx_cc_hook()
    part_name = (nc.partition_id_tensor.name
                 if nc.partition_id_tensor else None)
    in_names, out_names, out_avals, zero_outs = [], [], [], []
    for alloc in nc.m.functions[0].allocations:
        if not isinstance(alloc, mybir.MemoryLocationSet):
            continue
        name = alloc.memorylocations[0].name
        if alloc.kind == "ExternalInput":
            if name != part_name:
                in_names.append(name)
        elif alloc.kind == "ExternalOutput":
            out_names.append(name)
            shape = tuple(alloc.tensor_shape)
            dtype = mb.dt.np(alloc.dtype)
            out_avals.append(jax.core.ShapedArray(shape, dtype))
            zero_outs.append(np.zeros(shape, dtype))
    n_params = len(in_names)
    all_names = list(in_names) + list(out_names)
    if part_name is not None:
        all_names.append(part_name)

    def _body(*args):
        o = list(args[n_params:])
        for _ in range(reps):
            ops = list(args[:n_params]) + o
            if part_name is not None:
                ops.append(bass2jax.partition_id_tensor())
            o = list(bass2jax._bass_exec_p.bind(
                *ops,
                out_avals=tuple(out_avals),
                in_names=tuple(all_names),
                out_names=tuple(out_names),
                lowering_input_output_aliases=(),
                sim_require_finite=True,
                sim_require_nnan=True,
                nc=nc))
        return tuple(o)

    devices = jax.devices()[:NCORES]
    mesh = Mesh(np.asarray(devices), ("core",))
    n_outs = len(out_names)
    in_specs = (PartitionSpec("core"),) * (n_params + n_outs)
    out_specs = (PartitionSpec("core"),) * n_outs
    donate = tuple(range(n_params, n_params + n_outs))
    fn = jax.jit(
        shard_map(_body, mesh=mesh, in_specs=in_specs,
                  out_specs=out_specs, check_rep=False),
        donate_argnums=donate, keep_unused=True)
    return fn, mesh, in_names, out_names, out_avals, zero_outs


def _concat_inputs(in_maps, in_names):
    return [np.concatenate([np.asarray(in_maps[c][n]) for c in range(NCORES)],
                           axis=0) for n in in_names]


def _unpack_out(arrs, out_avals):
    # arrs[i]: [NCORES*dim0, ...] -> full [N,K,C] (inverting the window
    # rebalance permutation)
    xo = np.asarray(arrs[0]).astype(np.float32).reshape(NCORES, C, K, NLOCP)
    out = np.zeros((N, K, C), np.float32)
    perms = _PERMS["perms"]
    for d in range(NCORES):
        out[d * NLOC:(d + 1) * NLOC] = \
            xo[d][:, :, perms[d]].transpose(2, 1, 0)
    return out


def _run(TW, in_maps, reps_timing=0):
    """Returns (out, timing_info)."""
    import jax, time
    nc = _get_nc(TW)
    key = (TW, 1)
    if key not in _CACHE:
        _CACHE[key] = _make_runner(nc, 1)
    fn1, mesh, in_names, out_names, out_avals, zero_outs = _CACHE[key]
    cin = _concat_inputs(in_maps, in_names)
    czo = [np.zeros((NCORES * z.shape[0], *z.shape[1:]), z.dtype)
           for z in zero_outs]
    outs = fn1(*cin, *czo)
    jax.block_until_ready(outs)
    result = _unpack_out(outs, out_avals)

    timing = None
    if reps_timing:
        from jax.sharding import NamedSharding, PartitionSpec
        shard = NamedSharding(mesh, PartitionSpec("core"))
        cin_dev = [jax.device_put(a, shard) for a in cin]

        def one(f):
            z = [jax.device_put(np.zeros_like(a), shard) for a in czo]
            jax.block_until_ready(z)
            t0 = time.perf_counter()
            o = f(*cin_dev, *z)
            jax.block_until_ready(o)
            return time.perf_counter() - t0

        R = reps_timing
        if R > 1:
            ncR = _get_nc(TW, reps=R)
            kr = ("fn", TW, R)
            if kr not in _CACHE:
                _CACHE[kr] = _make_runner(ncR, 1)
            fnR = _CACHE[kr][0]
            # interleaved pairs: dispatch-floor drift is slow, so the
            # difference within an adjacent (fn1, fnR) pair isolates the
            # (R-1) extra kernel iterations; median rejects spikes.
            one(fn1), one(fnR), one(fn1), one(fnR)  # warm both
            ts1, tsR, diffs = [], [], []
            for _ in range(16):
                t1 = one(fn1)
                tR = one(fnR)
                ts1.append(t1)
                tsR.append(tR)
                diffs.append(tR - t1)
            diffs.sort()
            med = diffs[len(diffs) // 2]
            per_iter = med / (R - 1)
        else:
            one(fn1), one(fn1)
            ts1 = [one(fn1) for _ in range(8)]
            tsR = None
            per_iter = min(ts1)
        timing = dict(ts1=ts1, tsR=tsR, reps=R, per_iter=per_iter)
    return result, timing


_NOOP = {}


def _noop_floor(n):
    """Min wall-clock of a trivial 8-core bass program = dispatch floor."""
    import jax, time
    if "fn" not in _NOOP:
        nnc = bacc.Bacc("TRN2", target_bir_lowering=False, debug=False,
                        num_devices=NCORES)
        a_d = nnc.dram_tensor("a", [128, 128], F32, kind="ExternalInput")
        b_d = nnc.dram_tensor("b", [128, 128], F32, kind="ExternalOutput")
        with tile.TileContext(nnc) as ntc:
            with ntc.tile_pool(name="p", bufs=1) as p:
                t = p.tile([128, 128], F32)
                nnc.sync.dma_start(out=t[:], in_=a_d.ap())
                nnc.sync.dma_start(out=b_d.ap(), in_=t[:])
        nnc.compile()
        _NOOP["fn"] = _make_runner(nnc, 1)
    fn, mesh, in_names, out_names, out_avals, zero_outs = _NOOP["fn"]
    ain = np.zeros((NCORES * 128, 128), np.float32)
    ain_dev = jax.device_put(ain)
    best = float("inf")
    fn(ain_dev, jax.device_put(np.zeros_like(ain)))
    for _ in range(n):
        z = jax.device_put(np.zeros_like(ain))
        jax.block_until_ready(z)
        t0 = time.perf_counter()
        o = fn(ain_dev, z)
        jax.block_until_ready(o)
        best = min(best, time.perf_counter() - t0)
    return best


def kernel(**inputs):
    TW, in_maps = _prep(inputs)
    out, _ = _run(TW, in_maps)
    return out
